# revision 3
# baseline (speedup 1.0000x reference)
# Trainium2 Bass kernel for nn_DirectRanker (ragged_sequence).
#
# Math shortcut: result = tanh((sorted_enc[:,1:,:] - sorted_enc[:,:1,:]) @ W.T)
# commutes with the linear map, so per-row scores s = encodes @ W.T are
# computed FIRST (the memory-bound part: 512 MiB of fp16 streamed once), and
# the per-group sort/diff/tanh runs on the tiny [N] score vector:
#   result[g, k-1] = tanh(s_sorted[g, k] - s_sorted[g, 0]),  k = 1..63
#
# Sharding: groups split across 8 cores (2048 groups/core), no cross-core
# communication.
#
# v3: encodes is pre-arranged ON HOST to [2, 8, 128, 2, 8192] fp16
# (d-chunk, tile-pair, d, tile, col) so that every encode DMA is ONE dense
# 4 MiB linear HBM extent ([128, 16384] with partition stride 32 KiB) --
# linear extents measure 383 GB/s/core vs 308 for the strided layout.
# TensorE computes the matvec in its native orientation:
#   psum[1, 512] += W_chunk[128, 1].T @ ET_chunk[128, 512]
# (2 cycles/row on PE, no on-chip transposes).  PSUM exits to SBUF fp16
# alternate between ScalarE and VectorE so PE never stalls on a consumer
# (stalls would drop it into the HAM mid p-state at half clock).  A single
# SBUF->SBUF DMA per 128-group tile relayouts the flat score vector into
# [group(partition), elem(free)] for the sort.  All big streams ride the
# sync HWDGE ring; dependent small DMAs (score relayout, result store) ride
# the gpsimd SWDGE ring so they can't head-of-line block the encode stream.
#
# Exact stable argsort over y within each 64-row group: integer keys
#   key = (y * 2^23 + 2^23) * 64 | elem_index     (y is a multiple of 2^-23)
# are sorted through their f32 bitcast views (monotone for positive int32;
# keys lie in [2^29, 2^30) so the views are normal floats) with 8 rounds of
# DVE max8 + match_replace; perm = low 6 bits of the sorted keys. The score
# permutation runs on gpsimd local_scatter (fp16 as int16), ranks coming
# from scattering a descending iota by perm.
import os
from contextlib import ExitStack

import numpy as np

import concourse.bacc as bacc
import concourse.mybir as mybir
import concourse.tile as tile
from concourse.bass_utils import run_bass_kernel_spmd

N_CORES = 8
N = 1048576
D = 256
G = 64
NG = N // G                # 16384 groups
ROWS = N // N_CORES        # 131072 rows per core
GPC = NG // N_CORES        # 2048 groups per core
T_TILES = GPC // 128       # 16 tiles of 128 groups (8192 rows) per core
PAIRS = T_TILES // 2       # 8 DMA pairs (2 tiles per 4 MiB extent)
RPT = 128 * G              # rows per tile = 8192
MM_N = 512                 # moving free size per matmul (1 psum bank)
EXIT_N = 2048              # scores per psum exit copy (4 psum banks)
F32 = mybir.dt.float32
F16 = mybir.dt.float16
I32 = mybir.dt.int32
I16 = mybir.dt.int16
Alu = mybir.AluOpType
Act = mybir.ActivationFunctionType

_built = {}


def _build_nc():
    nc = bacc.Bacc("TRN2", target_bir_lowering=False, debug=False,
                   num_devices=N_CORES)
    # host-prearranged encodes: [d-chunk, pair, d, tile-in-pair, col]
    et_in = nc.dram_tensor("et", [2, PAIRS, 128, 2, RPT], F16,
                           kind="ExternalInput")
    y_in = nc.dram_tensor("y_coord", [ROWS], F32, kind="ExternalInput")
    w_in = nc.dram_tensor("w", [1, D], F32, kind="ExternalInput")
    out = nc.dram_tensor("result", [GPC * (G - 1)], F32, kind="ExternalOutput")

    out_r = out.ap().rearrange("(t p k) -> t p k", p=128, k=G - 1)

    with tile.TileContext(nc) as tc, ExitStack() as ctx:
        const_pool = ctx.enter_context(tc.tile_pool(name="const", bufs=1))
        epool = ctx.enter_context(tc.tile_pool(name="e", bufs=2))
        sfpool = ctx.enter_context(tc.tile_pool(name="sf", bufs=2))
        spool = ctx.enter_context(tc.tile_pool(name="s", bufs=3))
        scr_pool = ctx.enter_context(tc.tile_pool(name="scr", bufs=3))
        ps_pool = ctx.enter_context(
            tc.tile_pool(name="ps", bufs=2, space="PSUM"))

        # all of y for this core, loaded up-front (sync ring, before the
        # encode stream starts): y_all[p, T, u] = y[(T*128+p)*64 + u]
        y_all = const_pool.tile([128, T_TILES, G], F32)
        nc.sync.dma_start(
            y_all[:], y_in.ap().rearrange("(t p u) -> p t u", p=128, u=G))
        # W with d on partitions: wsb[:, c] = W[c*128:(c+1)*128]
        wsb = const_pool.tile([128, 2], F32)
        nc.sync.dma_start(wsb[:],
                          w_in.ap()[0, :].rearrange("(c p) -> p c", p=128))
        wsb_h = const_pool.tile([128, 2], F16)
        nc.vector.tensor_copy(wsb_h[:], wsb[:])
        # free-dim iota (elem index within group) for the sort keys
        iota_i = const_pool.tile([128, G], I32)
        nc.gpsimd.iota(iota_i[:], pattern=[[1, G]], base=0, channel_multiplier=0)
        # descending iota (63..0) as int16: data for the rank-producing scatter
        iota_d16 = const_pool.tile([128, G], I16)
        nc.gpsimd.iota(iota_d16[:], pattern=[[-1, G]], base=G - 1,
                       channel_multiplier=0)

        for pair in range(PAIRS):
            # two dense 4 MiB linear extents: all d for 2 tiles, per d-chunk
            etc0 = epool.tile([128, 2 * RPT], F16, tag="etc0")
            nc.sync.dma_start(
                etc0[:], et_in.ap()[0, pair].rearrange("p t n -> p (t n)"))
            etc1 = epool.tile([128, 2 * RPT], F16, tag="etc1")
            nc.sync.dma_start(
                etc1[:], et_in.ap()[1, pair].rearrange("p t n -> p (t n)"))

            for t in range(2):
                T = pair * 2 + t
                # ---- scores for tile T (native-orientation PE matvec) ----
                sflat = sfpool.tile([1, RPT], F16, tag="sflat")
                for j in range(RPT // EXIT_N):
                    ps = ps_pool.tile([1, EXIT_N], F32, tag="ps")
                    for q in range(EXIT_N // MM_N):
                        c0 = t * RPT + j * EXIT_N + q * MM_N
                        nc.tensor.matmul(ps[:, q * MM_N:(q + 1) * MM_N],
                                         wsb_h[:, 0:1], etc0[:, c0:c0 + MM_N],
                                         start=True, stop=False)
                        nc.tensor.matmul(ps[:, q * MM_N:(q + 1) * MM_N],
                                         wsb_h[:, 1:2], etc1[:, c0:c0 + MM_N],
                                         start=False, stop=True)
                    # exit psum -> sbuf fp16, alternating Act / DVE
                    dst = sflat[:, j * EXIT_N:(j + 1) * EXIT_N]
                    if j % 2 == 0:
                        nc.scalar.copy(dst, ps[:])
                    else:
                        nc.vector.tensor_copy(dst, ps[:])

                # ---- relayout flat scores -> [group(partition), elem] ----
                s_t = spool.tile([128, G], F16, tag="s")
                nc.gpsimd.dma_start(s_t[:], sflat[:])

                # ---- keys from y ----
                ki = spool.tile([128, G], I32, tag="ki")
                nc.scalar.activation(ki[:], y_all[:, T, :], Act.Copy,
                                     bias=float(1 << 23), scale=float(1 << 23))
                k64 = spool.tile([128, G], I32, tag="k64")
                nc.scalar.activation(k64[:], ki[:], Act.Copy,
                                     bias=0.0, scale=64.0)
                keys = spool.tile([128, G], I32, tag="keys")
                nc.vector.tensor_tensor(out=keys[:], in0=k64[:], in1=iota_i[:],
                                        op=Alu.bitwise_or)

                # ---- full descending sort of the int keys on DVE ----
                sorted_i = spool.tile([128, G], I32, tag="sorted")
                wka = scr_pool.tile([128, G], I32, tag="wka")
                wkb = scr_pool.tile([128, G], I32, tag="wkb")
                src = keys
                dst_t = wka
                for r in range(8):
                    nc.vector.max(sorted_i[:, r * 8:(r + 1) * 8].bitcast(F32),
                                  src[:].bitcast(F32))
                    if r < 7:
                        nc.vector.match_replace(
                            dst_t[:].bitcast(F32),
                            sorted_i[:, r * 8:(r + 1) * 8].bitcast(F32),
                            src[:].bitcast(F32), 0.0)
                        src, dst_t = dst_t, (wkb if dst_t is wka else wka)

                # perm (descending argsort) = low 6 bits of the sorted keys
                perm32 = scr_pool.tile([128, G], I32, tag="perm32")
                nc.vector.tensor_scalar(out=perm32[:], in0=sorted_i[:],
                                        scalar1=63, scalar2=None,
                                        op0=Alu.bitwise_and)
                perm16 = spool.tile([128, G], I16, tag="perm16")
                nc.scalar.copy(perm16[:], perm32[:])
                # rank_asc[i] = position of element i in ascending order
                rank16 = spool.tile([128, G], I16, tag="rank16")
                nc.gpsimd.local_scatter(rank16[:], iota_d16[:], perm16[:],
                                        channels=128, num_elems=G, num_idxs=G)

                # ---- permute fp16 scores by rank in one gpsimd scatter ----
                ssort = spool.tile([128, G], I16, tag="ssort")
                nc.gpsimd.local_scatter(ssort[:], s_t[:].bitcast(I16),
                                        rank16[:],
                                        channels=128, num_elems=G, num_idxs=G)
                ssf = ssort[:].bitcast(F16)

                # ---- result tile: tanh(ssort[:, 1:] - ssort[:, 0]) ----
                negs0 = spool.tile([128, 1], F32, tag="negs0")
                nc.scalar.mul(negs0[:], ssf[:, 0:1], -1.0)
                th = spool.tile([128, G - 1], F32, tag="th")
                nc.scalar.activation(th[:], ssf[:, 1:G], Act.Tanh,
                                     bias=negs0[:], scale=1.0)
                nc.gpsimd.dma_start(out_r[T], th[:])

    nc.compile()
    return nc


last_results = None


def kernel(encodes, y_coord, W, x_coord=None):
    global last_results
    if "nc" not in _built:
        _built["nc"] = _build_nc()
    nc = _built["nc"]

    e16 = np.asarray(encodes, dtype=np.float16)
    y_coord = np.ascontiguousarray(np.asarray(y_coord, dtype=np.float32))
    W = np.ascontiguousarray(np.asarray(W, dtype=np.float32))

    in_maps = []
    for c in range(N_CORES):
        # [ROWS, 256] -> [256, ROWS] -> [2, 128, 8, 2, 8192] (c d pair t n)
        # -> [2, 8, 128, 2, 8192] (c pair d t n), each [pair] slice one
        # dense 4 MiB extent per d-chunk
        et_c = np.ascontiguousarray(
            e16[c * ROWS:(c + 1) * ROWS].T
            .reshape(2, 128, PAIRS, 2, RPT)
            .transpose(0, 2, 1, 3, 4))
        in_maps.append({
            "et": et_c,
            "y_coord": y_coord[c * ROWS:(c + 1) * ROWS],
            "w": W,
        })
    # Only request tracing when the axon NTFF hook is importable; otherwise
    # force it off (bass_utils would crash importing antenv.axon_hooks if
    # BASS_TRACE leaked into the environment without the shim installed).
    want_trace = bool(os.environ.get("BASS_TRACE"))
    if want_trace:
        try:
            from antenv.axon_hooks import get_axon_ntff_profile_hook  # noqa: F401
        except ImportError:
            want_trace = False
            os.environ["BASS_NEVER_TRACE"] = "1"
    res = run_bass_kernel_spmd(
        nc, in_maps, core_ids=list(range(N_CORES)),
        trace=want_trace,
    )
    last_results = res
    result = np.concatenate([r["result"] for r in res.results])
    polarity = np.ones(NG * (G - 1), dtype=np.float32)
    return result, polarity
